# revision 27
# baseline (speedup 1.0000x reference)
"""Bidirectional Chamfer loss on 8 Trainium2 NeuronCores.

Math: for each batch pair (p, q):
    D[i, j] = ||p_i||^2 + ||q_j||^2 - 2 p_i . q_j
    cd = mean_i min_j D[i, j] + mean_j min_i D[i, j]
    loss = 0.7 * mean_b cd_filtered + 0.3 * mean_b cd_nonfiltered

Device computes -D (gt operands negated on host) so every min becomes a
max; host negates back at combine.

Mapping ("orientation B": gt points on PSUM partitions, pred rows on free):
  - Host packs, per (config, batch), K=24 bf16 matmul operands so one PE
    matmul emits -D tiles directly into PSUM. fp32 values are split 3-way
    into bf16 (hi, mid, lo) and products keep the 6 dominant cross terms
    -> error ~2^-27 per term.
  - pred rows sharded 8 ways (512/1024 rows per core); gt replicated.
  - PSUM tiles pair TWO gt chunks [128, 2R]: 4 matmuls fill, one ACT copy
    drains to bf16 SBUF (halves the fixed per-instruction ACT overhead).
  - Per tile:
      DVE  pm = max(chunkA, chunkB) elementwise (pred-side pair fold, @2x)
      Pool (gpsimd) pacc = max(pacc, pm): pred-side accumulate runs on the
           otherwise-idle engine
      DVE  fold1/2/3 per chunk (@2x bf16) -> 1/8-width residues staged;
           one grouped tensor_reduce per stage (alternating DVE/Pool)
           -> per-gt-point max over this core's pred rows
  - Epilogue: PE-transpose pacc in [128,128] chunks into a psum-ring tile,
    grouped free-axis reduce -> pred-side max per pred row.
  - Host: tiny cross-core max/mean combine, negate.
"""

import numpy as np

B = 4
NF = 4096
NN = 8192
NCORES = 8
RF = NF // NCORES   # 512 pred rows per core (filtered)
RN = NN // NCORES   # 1024 pred rows per core (nonfiltered)
K24 = 24            # contraction rows of the split-bf16 matmul

# output column layout
N_M1 = B * (RF // 128 + RN // 128)          # 4*(4+8) = 48   pred-side maxes
N_M2 = B * (NF // 128 + NN // 128)          # 4*(32+64) = 384 gt-side maxes

_CACHE = {}


def build_nc():
    """Build the per-core Bass program (SPMD: same program, different data)."""
    from contextlib import ExitStack

    import concourse.mybir as mybir
    import concourse.tile as tile
    from concourse import bacc

    f32 = mybir.dt.float32
    bf16 = mybir.dt.bfloat16
    Alu = mybir.AluOpType

    nc = bacc.Bacc("TRN2", target_bir_lowering=False, debug=False)

    # pred (sharded), gt (replicated) operands, [B, K24, n]
    Pf = nc.dram_tensor("pf", [B, K24, RF], bf16, kind="ExternalInput").ap()
    Gf = nc.dram_tensor("gf", [B, K24, NF], bf16, kind="ExternalInput").ap()
    Pn = nc.dram_tensor("pn", [B, K24, RN], bf16, kind="ExternalInput").ap()
    Gn = nc.dram_tensor("gn", [B, K24, NN], bf16, kind="ExternalInput").ap()
    O1 = nc.dram_tensor("m1", [128, N_M1], f32, kind="ExternalOutput").ap()
    O2 = nc.dram_tensor("m2", [128, N_M2], f32, kind="ExternalOutput").ap()

    with tile.TileContext(nc) as tc, ExitStack() as ctx:
        gpool = ctx.enter_context(tc.tile_pool(name="gt", bufs=2))
        ppool = ctx.enter_context(tc.tile_pool(name="pred", bufs=2))
        cpool = ctx.enter_context(tc.tile_pool(name="copy", bufs=3))
        pmpool = ctx.enter_context(tc.tile_pool(name="pm", bufs=2))
        f1pool = ctx.enter_context(tc.tile_pool(name="f1", bufs=2))
        f2pool = ctx.enter_context(tc.tile_pool(name="f2", bufs=2))
        stpool = ctx.enter_context(tc.tile_pool(name="stage", bufs=2))
        apool = ctx.enter_context(tc.tile_pool(name="pacc", bufs=1))
        tppool = ctx.enter_context(tc.tile_pool(name="tp", bufs=2))
        out_pool = ctx.enter_context(tc.tile_pool(name="outs", bufs=1))
        psum_pool = ctx.enter_context(tc.tile_pool(name="psum", bufs=2, space="PSUM"))

        m1t = out_pool.tile([128, N_M1], f32)
        m2t = out_pool.tile([128, N_M2], f32)

        m1col = 0
        m2col = 0
        for cfg, Pt, Gt, Npts, Rrows in (
            ("f", Pf, Gf, NF, RF),
            ("n", Pn, Gn, NN, RN),
        ):
            n_jt = Npts // 128       # gt chunks
            n_t = n_jt // 2          # 2-chunk tiles
            res = Rrows // 8         # residue width per chunk after 3 folds
            grp = 1024 // (2 * res)  # tiles per staged reduce
            n_mm = Rrows // 512      # matmuls (N<=512) per chunk
            for b in range(B):
                sG = gpool.tile([K24, Npts], bf16, tag="gt")
                nc.sync.dma_start(sG[:], Gt[b])
                sP = ppool.tile([K24, Rrows], bf16, tag="pred")
                nc.sync.dma_start(sP[:], Pt[b])
                pacc = apool.tile([128, Rrows], bf16, tag=f"pacc_{cfg}{b}")

                for t in range(n_t):
                    ps = psum_pool.tile([128, 2048], f32, tag="ps")
                    w = 2 * Rrows
                    for half in (0, 1):
                        jt = 2 * t + half
                        lhsT = sG[:, jt * 128 : (jt + 1) * 128]
                        for h in range(n_mm):
                            o = half * Rrows + h * 512
                            nc.tensor.matmul(
                                ps[:, o : o + min(512, Rrows)],
                                lhsT=lhsT,
                                rhs=sP[:, h * 512 : h * 512 + min(512, Rrows)],
                                start=True,
                                stop=True,
                            )
                    cp = cpool.tile([128, 2 * Rrows], bf16, tag="cp")
                    nc.scalar.copy(cp[:], ps[:, :w])

                    # pred-side: elementwise max of the two chunks, then
                    # accumulate into pacc on the gpsimd engine
                    if t == 0:
                        nc.vector.tensor_tensor(
                            out=pacc[:], in0=cp[:, :Rrows], in1=cp[:, Rrows:],
                            op=Alu.max,
                        )
                    else:
                        pm = pmpool.tile([128, Rrows], bf16, tag="pm")
                        nc.vector.tensor_tensor(
                            out=pm[:], in0=cp[:, :Rrows], in1=cp[:, Rrows:],
                            op=Alu.max,
                        )
                        nc.vector.tensor_tensor(
                            out=pacc[:], in0=pm[:], in1=pacc[:], op=Alu.max
                        )

                    # gt-side fold tree (both chunks per instruction, @2x)
                    f1 = f1pool.tile([128, Rrows], bf16, tag="f1")
                    v = cp[:].rearrange("p (c two h) -> p c two h", two=2, h=Rrows // 2)
                    nc.vector.tensor_tensor(
                        out=f1[:], in0=v[:, :, 0, :], in1=v[:, :, 1, :], op=Alu.max
                    )
                    f2 = f2pool.tile([128, Rrows // 2], bf16, tag="f2")
                    v = f1[:].rearrange("p (c two h) -> p c two h", two=2, h=Rrows // 4)
                    nc.vector.tensor_tensor(
                        out=f2[:], in0=v[:, :, 0, :], in1=v[:, :, 1, :], op=Alu.max
                    )
                    slot = t % grp
                    if slot == 0:
                        stg = stpool.tile([128, 1024], bf16, tag="stage")
                    v = f2[:].rearrange("p (c two h) -> p c two h", two=2, h=Rrows // 8)
                    nc.vector.tensor_tensor(
                        out=stg[:, slot * 2 * res : (slot + 1) * 2 * res],
                        in0=v[:, :, 0, :],
                        in1=v[:, :, 1, :],
                        op=Alu.max,
                    )
                    if slot == grp - 1:
                        c0 = m2col + 2 * (t - slot)
                        nc.vector.tensor_reduce(
                            out=m2t[:, c0 : c0 + 2 * grp],
                            in_=stg[:].rearrange("p (g k) -> p g k", k=res),
                            axis=mybir.AxisListType.X,
                            op=Alu.max,
                        )
                m2col += n_jt

                # epilogue: pred-side max over the 128 gt lanes of pacc:
                # XBAR DMA-transpose [128,128] blocks, then grouped reduce
                n_ch = Rrows // 128
                tp = tppool.tile([128, Rrows], bf16, tag="tp")
                for c in range(n_ch):
                    nc.sync.dma_start_transpose(
                        tp[:, c * 128 : (c + 1) * 128],
                        pacc[:, c * 128 : (c + 1) * 128],
                    )
                nc.vector.tensor_reduce(
                    out=m1t[:, m1col : m1col + n_ch],
                    in_=tp[:].rearrange("p (c k) -> p c k", k=128),
                    axis=mybir.AxisListType.X,
                    op=Alu.max,
                )
                m1col += n_ch

        nc.sync.dma_start(O1[:], m1t[:])
        nc.sync.dma_start(O2[:], m2t[:])

    nc.compile()
    return nc


def _split3(x):
    """fp32 -> three bf16 arrays with x ~= b0+b1+b2 (error ~2^-27 |x|)."""
    import ml_dtypes

    bf = ml_dtypes.bfloat16
    b0 = x.astype(bf)
    r1 = (x - b0.astype(np.float32)).astype(np.float32)
    b1 = r1.astype(bf)
    r2 = (r1 - b1.astype(np.float32)).astype(np.float32)
    b2 = r2.astype(bf)
    return b0, b1, b2


# product-pair pattern per coordinate: (gt split idx, pred split idx)
_PAIRS = ((0, 0), (0, 1), (1, 0), (0, 2), (1, 1), (2, 0))


def pack_inputs(pred_filtered, gt_filtered, pred_nonfiltered, gt_nonfiltered):
    """Build per-core input maps (bf16 split operands for -D)."""
    import ml_dtypes

    bf = ml_dtypes.bfloat16

    def mk(p, q):
        # returns (P [B,24,Np] , G [B,24,Nq]) bf16 such that G.T @ P = -D
        p = p.astype(np.float32)
        q = q.astype(np.float32)
        Bn, Np_, _ = p.shape
        Nq = q.shape[1]
        P = np.zeros((Bn, K24, Np_), bf)
        G = np.zeros((Bn, K24, Nq), bf)
        pp = np.sum(p * p, axis=-1, dtype=np.float32)
        qq = np.sum(q * q, axis=-1, dtype=np.float32)
        for c in range(3):
            ws = _split3(2.0 * p[..., c])      # pred-side coord splits (+2p)
            gs = _split3(q[..., c])            # gt-side coord splits
            for t, (gi, wi) in enumerate(_PAIRS):
                G[:, 6 * c + t, :] = gs[gi]
                P[:, 6 * c + t, :] = ws[wi]
        qqs = _split3(-qq)
        pps = _split3(-pp)
        for t in range(3):
            G[:, 18 + t, :] = qqs[t]
            P[:, 18 + t, :] = np.ones_like(pp, dtype=bf)
            G[:, 21 + t, :] = np.ones_like(qq, dtype=bf)
            P[:, 21 + t, :] = pps[t]
        return P, G

    pf_all, gf = mk(pred_filtered, gt_filtered)
    pn_all, gn = mk(pred_nonfiltered, gt_nonfiltered)
    gf = np.ascontiguousarray(gf)
    gn = np.ascontiguousarray(gn)

    in_maps = []
    for k in range(NCORES):
        in_maps.append(
            {
                "pf": np.ascontiguousarray(pf_all[:, :, k * RF : (k + 1) * RF]),
                "gf": gf,
                "pn": np.ascontiguousarray(pn_all[:, :, k * RN : (k + 1) * RN]),
                "gn": gn,
            }
        )
    return in_maps


def combine_outputs(results):
    """results: list (per core) of {"m1": [128,48], "m2": [128,384]} -> loss.

    Device values are maxes of -D; negate at the end.
    """
    cds = {}
    for cfg, (Npts, Rrows, m1off, m2off) in (
        ("f", (NF, RF, 0, 0)),
        ("n", (NN, RN, B * (RF // 128), B * (NF // 128))),
    ):
        n_ch = Rrows // 128   # m1 cols per batch (pred rows / 128, per core)
        n_jt = Npts // 128    # m2 cols per batch (gt chunks)
        # pred-side: values are per-pred-row maxes already; mean over all
        m1 = np.stack(
            [r["m1"][:, m1off : m1off + B * n_ch] for r in results], axis=0
        ).reshape(NCORES, 128, B, n_ch)
        pred_mean = -m1.mean(axis=(0, 1, 3))  # [B]
        # gt-side: per-core partial maxes -> max across cores, mean over gt
        m2 = np.stack(
            [r["m2"][:, m2off : m2off + B * n_jt] for r in results], axis=0
        )
        m2 = m2.max(axis=0).reshape(128, B, n_jt)
        gt_mean = -m2.mean(axis=(0, 2))  # [B]
        cds[cfg] = (pred_mean + gt_mean).mean()
    return np.float32(0.7 * cds["f"] + 0.3 * cds["n"])


def kernel(pred_filtered, gt_filtered, pred_nonfiltered, gt_nonfiltered):
    from concourse.bass_utils import run_bass_kernel_spmd

    if "nc" not in _CACHE:
        _CACHE["nc"] = build_nc()
    in_maps = pack_inputs(
        pred_filtered, gt_filtered, pred_nonfiltered, gt_nonfiltered
    )
    res = run_bass_kernel_spmd(_CACHE["nc"], in_maps, core_ids=list(range(NCORES)))
    return combine_outputs(res.results)


# revision 30
# speedup vs baseline: 1.0499x; 1.0499x over previous
"""Bidirectional Chamfer loss on 8 Trainium2 NeuronCores.

Math: for each batch pair (p, q):
    D[i, j] = ||p_i||^2 + ||q_j||^2 - 2 p_i . q_j
    cd = mean_i min_j D[i, j] + mean_j min_i D[i, j]
    loss = 0.7 * mean_b cd_filtered + 0.3 * mean_b cd_nonfiltered

Device computes -D (gt operands negated on host) so every min becomes a
max; host negates back at combine.

Mapping ("orientation B": gt points on PSUM partitions, pred rows on free):
  - Host packs, per (config, batch), K=24 bf16 matmul operands so one PE
    matmul emits -D tiles directly into PSUM. fp32 values are split 3-way
    into bf16 (hi, mid, lo) and products keep the 6 dominant cross terms
    -> error ~2^-27 per term.
  - pred rows sharded 8 ways (512/1024 rows per core); gt replicated.
  - PSUM tiles pair TWO gt chunks [128, 2R]: 4 matmuls fill, one ACT copy
    drains to bf16 SBUF (halves the fixed per-instruction ACT overhead).
  - Per tile:
      DVE  pm = max(chunkA, chunkB) elementwise (pred-side pair fold, @2x)
      Pool (gpsimd) pacc = max(pacc, pm): pred-side accumulate runs on the
           otherwise-idle engine
      DVE  fold1/2/3 per chunk (@2x bf16) -> 1/8-width residues staged;
           one grouped tensor_reduce per stage (alternating DVE/Pool)
           -> per-gt-point max over this core's pred rows
  - Epilogue: PE-transpose pacc in [128,128] chunks into a psum-ring tile,
    grouped free-axis reduce -> pred-side max per pred row.
  - Host: tiny cross-core max/mean combine, negate.
"""

import numpy as np

B = 4
NF = 4096
NN = 8192
NCORES = 8
RF = NF // NCORES   # 512 pred rows per core (filtered)
RN = NN // NCORES   # 1024 pred rows per core (nonfiltered)
K24 = 24            # contraction rows of the split-bf16 matmul

# output column layout
N_M1 = B * (RF // 128 + RN // 128)          # 4*(4+8) = 48   pred-side maxes
N_M2 = B * (NF // 128 + NN // 128)          # 4*(32+64) = 384 gt-side maxes

_CACHE = {}


def build_nc():
    """Build the per-core Bass program (SPMD: same program, different data)."""
    from contextlib import ExitStack

    import concourse.mybir as mybir
    import concourse.tile as tile
    from concourse import bacc

    f32 = mybir.dt.float32
    bf16 = mybir.dt.bfloat16
    Alu = mybir.AluOpType

    nc = bacc.Bacc("TRN2", target_bir_lowering=False, debug=False)

    # pred (sharded), gt (replicated) operands, [B, K24, n]
    Pf = nc.dram_tensor("pf", [B, K24, RF], bf16, kind="ExternalInput").ap()
    Gf = nc.dram_tensor("gf", [B, K24, NF], bf16, kind="ExternalInput").ap()
    Pn = nc.dram_tensor("pn", [B, K24, RN], bf16, kind="ExternalInput").ap()
    Gn = nc.dram_tensor("gn", [B, K24, NN], bf16, kind="ExternalInput").ap()
    O1 = nc.dram_tensor("m1", [128, N_M1], f32, kind="ExternalOutput").ap()
    O2 = nc.dram_tensor("m2", [128, N_M2], f32, kind="ExternalOutput").ap()

    with tile.TileContext(nc) as tc, ExitStack() as ctx:
        gpool = ctx.enter_context(tc.tile_pool(name="gt", bufs=2))
        ppool = ctx.enter_context(tc.tile_pool(name="pred", bufs=2))
        cpool = ctx.enter_context(tc.tile_pool(name="copy", bufs=3))
        pmpool = ctx.enter_context(tc.tile_pool(name="pm", bufs=2))
        pppool = ctx.enter_context(tc.tile_pool(name="pp", bufs=2))
        f1pool = ctx.enter_context(tc.tile_pool(name="f1", bufs=2))
        f2pool = ctx.enter_context(tc.tile_pool(name="f2", bufs=2))
        stpool = ctx.enter_context(tc.tile_pool(name="stage", bufs=2))
        apool = ctx.enter_context(tc.tile_pool(name="pacc", bufs=1))
        tppool = ctx.enter_context(tc.tile_pool(name="tp", bufs=2))
        out_pool = ctx.enter_context(tc.tile_pool(name="outs", bufs=1))
        psum_pool = ctx.enter_context(tc.tile_pool(name="psum", bufs=2, space="PSUM"))

        m1t = out_pool.tile([128, N_M1], f32)
        m2t = out_pool.tile([128, N_M2], f32)

        m1col = 0
        m2col = 0
        for cfg, Pt, Gt, Npts, Rrows in (
            ("f", Pf, Gf, NF, RF),
            ("n", Pn, Gn, NN, RN),
        ):
            n_jt = Npts // 128       # gt chunks
            n_t = n_jt // 2          # 2-chunk tiles
            res = Rrows // 8         # residue width per chunk after 3 folds
            grp = 1024 // (4 * res)  # tile-pairs per staged reduce
            n_mm = Rrows // 512      # matmuls (N<=512) per chunk
            for b in range(B):
                sG = gpool.tile([K24, Npts], bf16, tag="gt")
                nc.sync.dma_start(sG[:], Gt[b])
                sP = ppool.tile([K24, Rrows], bf16, tag="pred")
                nc.sync.dma_start(sP[:], Pt[b])
                pacc = apool.tile([128, Rrows], bf16, tag=f"pacc_{cfg}{b}")

                # PAIRS of psum tiles (4 gt chunks) share each DVE op group
                for u in range(n_t // 2):
                    cp = cpool.tile([128, 4 * Rrows], bf16, tag="cp")
                    for ti in (0, 1):
                        ps = psum_pool.tile([128, 2048], f32, tag="ps")
                        for half in (0, 1):
                            jt = 4 * u + 2 * ti + half
                            lhsT = sG[:, jt * 128 : (jt + 1) * 128]
                            for h in range(n_mm):
                                o = half * Rrows + h * 512
                                nc.tensor.matmul(
                                    ps[:, o : o + min(512, Rrows)],
                                    lhsT=lhsT,
                                    rhs=sP[:, h * 512 : h * 512 + min(512, Rrows)],
                                    start=True,
                                    stop=True,
                                )
                        co = ti * 2 * Rrows
                        nc.scalar.copy(cp[:, co : co + 2 * Rrows], ps[:, : 2 * Rrows])

                    # pred-side: 4-chunk tree -> pacc (all @2x bf16)
                    pm = pmpool.tile([128, 2 * Rrows], bf16, tag="pm")
                    nc.vector.tensor_tensor(
                        out=pm[:], in0=cp[:, : 2 * Rrows], in1=cp[:, 2 * Rrows :],
                        op=Alu.max,
                    )
                    if u == 0:
                        nc.vector.tensor_tensor(
                            out=pacc[:], in0=pm[:, :Rrows], in1=pm[:, Rrows:],
                            op=Alu.max,
                        )
                    else:
                        pp = pppool.tile([128, Rrows], bf16, tag="pp")
                        nc.vector.tensor_tensor(
                            out=pp[:], in0=pm[:, :Rrows], in1=pm[:, Rrows:],
                            op=Alu.max,
                        )
                        nc.vector.tensor_tensor(
                            out=pacc[:], in0=pp[:], in1=pacc[:], op=Alu.max
                        )

                    # gt-side fold tree (all 4 chunks per instruction, @2x)
                    f1 = f1pool.tile([128, 2 * Rrows], bf16, tag="f1")
                    v = cp[:].rearrange(
                        "p (c two h) -> p c two h", two=2, h=Rrows // 2
                    )
                    nc.vector.tensor_tensor(
                        out=f1[:], in0=v[:, :, 0, :], in1=v[:, :, 1, :], op=Alu.max
                    )
                    f2 = f2pool.tile([128, Rrows], bf16, tag="f2")
                    v = f1[:].rearrange(
                        "p (c two h) -> p c two h", two=2, h=Rrows // 4
                    )
                    nc.vector.tensor_tensor(
                        out=f2[:], in0=v[:, :, 0, :], in1=v[:, :, 1, :], op=Alu.max
                    )
                    slot = u % grp
                    if slot == 0:
                        stg = stpool.tile([128, 1024], bf16, tag="stage")
                    v = f2[:].rearrange(
                        "p (c two h) -> p c two h", two=2, h=Rrows // 8
                    )
                    nc.vector.tensor_tensor(
                        out=stg[:, slot * 4 * res : (slot + 1) * 4 * res],
                        in0=v[:, :, 0, :],
                        in1=v[:, :, 1, :],
                        op=Alu.max,
                    )
                    if slot == grp - 1:
                        c0 = m2col + 4 * (u - slot)
                        nc.vector.tensor_reduce(
                            out=m2t[:, c0 : c0 + 4 * grp],
                            in_=stg[:].rearrange("p (g k) -> p g k", k=res),
                            axis=mybir.AxisListType.X,
                            op=Alu.max,
                        )
                m2col += n_jt

                # epilogue: pred-side max over the 128 gt lanes of pacc:
                # XBAR DMA-transpose [128,128] blocks, then grouped reduce
                n_ch = Rrows // 128
                tp = tppool.tile([128, Rrows], bf16, tag="tp")
                for c in range(n_ch):
                    nc.sync.dma_start_transpose(
                        tp[:, c * 128 : (c + 1) * 128],
                        pacc[:, c * 128 : (c + 1) * 128],
                    )
                nc.vector.tensor_reduce(
                    out=m1t[:, m1col : m1col + n_ch],
                    in_=tp[:].rearrange("p (c k) -> p c k", k=128),
                    axis=mybir.AxisListType.X,
                    op=Alu.max,
                )
                m1col += n_ch

        nc.sync.dma_start(O1[:], m1t[:])
        nc.sync.dma_start(O2[:], m2t[:])

    nc.compile()
    return nc


def _split3(x):
    """fp32 -> three bf16 arrays with x ~= b0+b1+b2 (error ~2^-27 |x|)."""
    import ml_dtypes

    bf = ml_dtypes.bfloat16
    b0 = x.astype(bf)
    r1 = (x - b0.astype(np.float32)).astype(np.float32)
    b1 = r1.astype(bf)
    r2 = (r1 - b1.astype(np.float32)).astype(np.float32)
    b2 = r2.astype(bf)
    return b0, b1, b2


# product-pair pattern per coordinate: (gt split idx, pred split idx)
_PAIRS = ((0, 0), (0, 1), (1, 0), (0, 2), (1, 1), (2, 0))


def pack_inputs(pred_filtered, gt_filtered, pred_nonfiltered, gt_nonfiltered):
    """Build per-core input maps (bf16 split operands for -D)."""
    import ml_dtypes

    bf = ml_dtypes.bfloat16

    def mk(p, q):
        # returns (P [B,24,Np] , G [B,24,Nq]) bf16 such that G.T @ P = -D
        p = p.astype(np.float32)
        q = q.astype(np.float32)
        Bn, Np_, _ = p.shape
        Nq = q.shape[1]
        P = np.zeros((Bn, K24, Np_), bf)
        G = np.zeros((Bn, K24, Nq), bf)
        pp = np.sum(p * p, axis=-1, dtype=np.float32)
        qq = np.sum(q * q, axis=-1, dtype=np.float32)
        for c in range(3):
            ws = _split3(2.0 * p[..., c])      # pred-side coord splits (+2p)
            gs = _split3(q[..., c])            # gt-side coord splits
            for t, (gi, wi) in enumerate(_PAIRS):
                G[:, 6 * c + t, :] = gs[gi]
                P[:, 6 * c + t, :] = ws[wi]
        qqs = _split3(-qq)
        pps = _split3(-pp)
        for t in range(3):
            G[:, 18 + t, :] = qqs[t]
            P[:, 18 + t, :] = np.ones_like(pp, dtype=bf)
            G[:, 21 + t, :] = np.ones_like(qq, dtype=bf)
            P[:, 21 + t, :] = pps[t]
        return P, G

    pf_all, gf = mk(pred_filtered, gt_filtered)
    pn_all, gn = mk(pred_nonfiltered, gt_nonfiltered)
    gf = np.ascontiguousarray(gf)
    gn = np.ascontiguousarray(gn)

    in_maps = []
    for k in range(NCORES):
        in_maps.append(
            {
                "pf": np.ascontiguousarray(pf_all[:, :, k * RF : (k + 1) * RF]),
                "gf": gf,
                "pn": np.ascontiguousarray(pn_all[:, :, k * RN : (k + 1) * RN]),
                "gn": gn,
            }
        )
    return in_maps


def combine_outputs(results):
    """results: list (per core) of {"m1": [128,48], "m2": [128,384]} -> loss.

    Device values are maxes of -D; negate at the end.
    """
    cds = {}
    for cfg, (Npts, Rrows, m1off, m2off) in (
        ("f", (NF, RF, 0, 0)),
        ("n", (NN, RN, B * (RF // 128), B * (NF // 128))),
    ):
        n_ch = Rrows // 128   # m1 cols per batch (pred rows / 128, per core)
        n_jt = Npts // 128    # m2 cols per batch (gt chunks)
        # pred-side: values are per-pred-row maxes already; mean over all
        m1 = np.stack(
            [r["m1"][:, m1off : m1off + B * n_ch] for r in results], axis=0
        ).reshape(NCORES, 128, B, n_ch)
        pred_mean = -m1.mean(axis=(0, 1, 3))  # [B]
        # gt-side: per-core partial maxes -> max across cores, mean over gt
        m2 = np.stack(
            [r["m2"][:, m2off : m2off + B * n_jt] for r in results], axis=0
        )
        m2 = m2.max(axis=0).reshape(128, B, n_jt)
        gt_mean = -m2.mean(axis=(0, 2))  # [B]
        cds[cfg] = (pred_mean + gt_mean).mean()
    return np.float32(0.7 * cds["f"] + 0.3 * cds["n"])


def kernel(pred_filtered, gt_filtered, pred_nonfiltered, gt_nonfiltered):
    from concourse.bass_utils import run_bass_kernel_spmd

    if "nc" not in _CACHE:
        _CACHE["nc"] = build_nc()
    in_maps = pack_inputs(
        pred_filtered, gt_filtered, pred_nonfiltered, gt_nonfiltered
    )
    res = run_bass_kernel_spmd(_CACHE["nc"], in_maps, core_ids=list(range(NCORES)))
    return combine_outputs(res.results)
